# revision 22
# baseline (speedup 1.0000x reference)
"""Multi-head self-attention Trainium2 Bass kernel (8-core SPMD).

Sharding: tensor-parallel over (batch, head-pair). With B=2 batches and
H=8 heads there are exactly 8 (batch, head-pair) units; core c handles
batch c//4 and heads {2*(c%4), 2*(c%4)+1}. Each core computes Q/K/V for its
two heads over the full sequence, runs attention, and produces the partial
output projection O_pair @ Wo_pair (no bias). The host sums the four
partials per batch and adds the output bias — a cheap numpy reduction.
Per-core weight slices are passed as separate inputs so the program stays
SPMD-uniform.

Layout strategy: activations live transposed in SBUF ([D, S], d on
partitions). Projections then need no weight transposes:
  K^T = Wk^T x^T   (lhsT = Wk chunk, rhs = x^T chunk)
  V   = x Wv       (lhsT = x^T chunk, rhs = Wv chunk)
Scores are computed transposed ([k, q], k on partitions) so softmax's
denominator comes from a ones-column appended to V (row 64 of the attention
output accumulator), and A^T is directly consumable by the A@V matmul.
exp() runs on the scalar engine with the 1/sqrt(dk) folded into its scale.
The normalized per-head outputs O^T are exactly the lhsT the output
projection wants, so no transposes are needed anywhere except on the input x.

Matmul operands are stored as fp16 (10-bit mantissa; measured end-to-end
absmax relative error ~4e-4): this is the true MAC path, so the PE
clock-gate can warm to 2.4 GHz and fast weight load applies. All
accumulation is fp32 in PSUM; softmax denominators/reciprocals are fp32.

The two heads' score matmuls share one [128,1024] PSUM tile and are pinned
adjacent via a scheduler dependency edge, so they stream through disjoint
PE row strips (0-63 / 64-127) concurrently; one exp() covers both. A@V
matmuls lag three k-tiles behind the scores so their exp() inputs are
always ready.
"""

from contextlib import ExitStack

import numpy as np

import concourse.bass as bass
import concourse.tile as tile
from concourse import bacc, mybir
from concourse.bass import _add_dep_helper
from concourse.bass_utils import run_bass_kernel_spmd

N_CORES = 8
B, S, D, H, DK = 2, 4096, 512, 8, 64
P = 128
NT_S = S // P                  # 32 sequence tiles
NT_D = D // P                  # 4 d-model chunks
QC = S // 512                  # 8 query chunks of 512
VW = 2 * 65                    # 130: per-k-tile width of the augmented V
F32 = mybir.dt.float32
F32R = mybir.dt.float32r
F16 = mybir.dt.float16
EXP = mybir.ActivationFunctionType.Exp

# "f16" (10 mantissa bits, 2.4 GHz MAC path + FWL), "f32r" (13 bits but
# pinned at the 1.2 GHz throttled clock), "f32" (exact, 4 cycles/row).
MM_DTYPE = "f16"
DTM = {"f32r": F32R, "f16": F16, "f32": F32}[MM_DTYPE]


def _emit(ctx: ExitStack, tc: tile.TileContext, io: dict):
    nc = tc.nc
    xb = io["xb"]
    wqp, wkp, wvp, wop = io["wqp"], io["wkp"], io["wvp"], io["wop"]
    bqp, bkp, bvp = io["bqp"], io["bkp"], io["bvp"]
    ident = io["ident"]
    out = io["out"]

    mm = nc.tensor.matmul

    # ---- pools ------------------------------------------------------------
    consts = ctx.enter_context(tc.tile_pool(name="consts", bufs=1))
    xt_pool = ctx.enter_context(tc.tile_pool(name="xt", bufs=1))
    qt_pool = ctx.enter_context(tc.tile_pool(name="qt", bufs=1))
    kt_pool = ctx.enter_context(tc.tile_pool(name="kt", bufs=1))
    v_pool = ctx.enter_context(tc.tile_pool(name="v", bufs=1))
    ot_pool = ctx.enter_context(tc.tile_pool(name="ot", bufs=2))
    w_pool = ctx.enter_context(tc.tile_pool(name="w", bufs=1))
    stg = ctx.enter_context(tc.tile_pool(name="stg", bufs=3))
    e_pool = ctx.enter_context(tc.tile_pool(name="e", bufs=8))
    rc_pool = ctx.enter_context(tc.tile_pool(name="rc", bufs=4))
    y_pool = ctx.enter_context(tc.tile_pool(name="y", bufs=3))
    # PSUM: shared [128,1024] pool (3 bufs x 2 banks) + attention
    # accumulators (2 banks). Projections use [0:512] slices of the pool.
    ps_pool = ctx.enter_context(tc.tile_pool(name="ps", bufs=3, space="PSUM"))
    o_pool = ctx.enter_context(tc.tile_pool(name="o", bufs=2, space="PSUM"))

    def psum1024(dt=F32):
        return ps_pool.tile([P, 1024], dt, tag="ps", name="ps")

    def psum512(dt=F32):
        return psum1024(dt)[:, 0:512]

    # ---- constants --------------------------------------------------------
    ident_sb = consts.tile([P, P], F32, tag="ident")
    nc.sync.dma_start(out=ident_sb[:], in_=ident[:])
    ones_f32 = consts.tile([P, 1], F32, tag="ones_f32")
    nc.vector.memset(ones_f32[:], 1.0)
    ones_sb = consts.tile([1, P], DTM, tag="ones")
    nc.vector.tensor_copy(out=ones_sb[:], in_=ones_f32[0:1, 0:1].broadcast_to([1, P]))
    # a f32 ones row living on partition 64 (denominator broadcast lhsT)
    ones64_sb = consts.tile([65, 64], F32, tag="ones64")
    nc.vector.memset(ones64_sb[64:65, :], 1.0)
    # per-partition bias columns for K^T/Q^T (fused into the PSUM->SBUF
    # copies); bv as a [1, 128] row for the rank-1 bias matmul.
    bkT = consts.tile([P, 1], F32, tag="bkT")
    nc.sync.dma_start(out=bkT[:], in_=bkp[:])
    bqT = consts.tile([P, 1], F32, tag="bqT")
    nc.sync.dma_start(out=bqT[:], in_=bqp[:])
    bv_st = consts.tile([1, P], F32, tag="bv_st")
    nc.sync.dma_start(out=bv_st[:], in_=bvp[:])
    bv_sb = consts.tile([1, P], DTM, tag="bv")
    nc.vector.tensor_copy(out=bv_sb[:], in_=bv_st[:])

    # per-core weight slices -> fp16 SBUF tiles
    def load_w(ap, rows, cols, tag):
        st = stg.tile([P, (rows // P) * cols], F32, tag="wstg")
        nc.sync.dma_start(
            out=st[:, :].rearrange("p (dc m) -> p dc m", dc=rows // P),
            in_=ap.rearrange("(dc p) m -> p dc m", p=P),
        )
        t = w_pool.tile([P, (rows // P) * cols], DTM, tag=tag)
        nc.vector.tensor_copy(out=t[:], in_=st[:])
        return t

    # x^T, Q^T, K^T are held as 4 sequence-quarter tiles so dependency
    # tracking (whole-tile granularity) lets projections start as soon as
    # the quarter they need is transposed, and attention as soon as the
    # first K/Q quarters exist.
    SQ = S // 4                 # 1024 columns per quarter
    xTq = [xt_pool.tile([P, NT_D * SQ], DTM, tag="xT", name=f"xT{i}",
                        bufs=4) for i in range(4)]

    def xslice(dc, s0, s1):
        i = s0 // SQ
        return xTq[i][:, dc * SQ + s0 - i * SQ: dc * SQ + s1 - i * SQ]

    # ---- stage A: x^T via PE transposes ----------------------------------
    with tc.tile_pool(name="xn", bufs=6) as xn_pool:
        for st in range(NT_S):
            xn = xn_pool.tile([P, D], F32, tag="xn")
            nc.sync.dma_start(out=xn[:], in_=xb[st * P:(st + 1) * P, :])
            tp = psum512()
            for dc in range(NT_D):
                nc.tensor.transpose(
                    tp[:, dc * P:(dc + 1) * P],
                    xn[:, dc * P:(dc + 1) * P],
                    ident_sb[:],
                )
            dst = xTq[st // 8]
            dst_ap = dst[:, :].rearrange("p (dc s) -> p dc s", dc=NT_D)
            so = (st % 8) * P
            nc.vector.tensor_copy(
                out=dst_ap[:, :, so:so + P],
                in_=tp[:, :].rearrange("p (dc j) -> p dc j", dc=NT_D),
            )

    # ---- stage B: Q^T, K^T, V for the pair -------------------------------
    wq_sb = load_w(wqp, D, P, "wq")
    wk_sb = load_w(wkp, D, P, "wk")
    qtq = [qt_pool.tile([P, SQ], DTM, tag="QT", name=f"QT{i}", bufs=4)
           for i in range(4)]
    ktq = [kt_pool.tile([P, SQ], DTM, tag="KT", name=f"KT{i}", bufs=4)
           for i in range(4)]
    for sc in range(QC):
        for w_sb, dstq, bT in ((wk_sb, ktq, bkT), (wq_sb, qtq, bqT)):
            ps = psum512()
            for dc in range(NT_D):
                mm(ps[:], w_sb[:, dc * P:(dc + 1) * P],
                   xslice(dc, sc * 512, (sc + 1) * 512),
                   start=(dc == 0), stop=(dc == NT_D - 1))
            so = (sc % 2) * 512
            nc.vector.tensor_scalar_add(
                out=dstq[sc // 2][:, so:so + 512], in0=ps[:], scalar1=bT[:],
            )
    # V (2 heads) with a ones column per head:
    # vaug[:, kt*130 + hl*65 + (0..63)] = V[k-tile, head hl]; [.. + 64] = 1
    wv_sb = load_w(wvp, D, P, "wv")
    vaug = v_pool.tile([P, NT_S * VW], DTM, tag="vaug")
    nc.vector.tensor_copy(
        out=vaug[:, :].rearrange("p (t h e) -> p t h e",
                                 t=NT_S, h=2)[:, :, :, 64:65],
        in_=ones_f32[:, 0:1].broadcast_to([P, NT_S, 2, 1]),
    )
    for st in range(NT_S):
        ps = psum512()
        for dc in range(NT_D):
            mm(ps[:, 0:P], xslice(dc, st * P, (st + 1) * P),
               wv_sb[:, dc * P:(dc + 1) * P],
               start=(dc == 0), stop=False)
        mm(ps[:, 0:P], ones_sb[0:1, :], bv_sb[0:1, :],
           start=False, stop=True)
        dst = vaug[:, st * VW:(st + 1) * VW]
        dst = dst.rearrange("p (h e) -> p h e", h=2)[:, :, 0:64]
        nc.vector.tensor_copy(
            out=dst, in_=ps[:, 0:P].rearrange("p (h e) -> p h e", h=2)
        )

    # ---- stage C: attention ----------------------------------------------
    ot0 = ot_pool.tile([64, S], DTM, tag="OT")
    ot1 = ot_pool.tile([64, S], DTM, tag="OT")
    for qc in range(QC):
        qsl = slice(qc * 512, (qc + 1) * 512)
        o0 = o_pool.tile([65, 512], F32, tag="O")
        o1 = o_pool.tile([65, 512], F32, tag="O")

        def emit_av(ktile, ea, gate):
            st_ = ktile * VW
            fl = dict(start=(ktile == 0), stop=(ktile == NT_S - 1))
            i0 = mm(o0[:], vaug[:, st_ + 0 * 65:st_ + 0 * 65 + 65],
                    ea[:, 0:512], **fl)
            i1 = mm(o1[:], vaug[:, st_ + 1 * 65:st_ + 1 * 65 + 65],
                    ea[:, 512:1024], **fl)
            if gate is not None:
                # order A@V after the next score pair: keeps the paired
                # heads adjacent in the PE stream
                _add_dep_helper(i0.ins, gate.ins, sync=False,
                                reason="attn pipeline order")
                _add_dep_helper(i1.ins, gate.ins, sync=False,
                                reason="attn pipeline order")

        qq = qtq[qc // 2]
        qlo = (qc % 2) * 512
        qls = slice(qlo, qlo + 512)
        pending = []  # [(ktile, ea), ...] not yet AV-emitted
        for ktile in range(NT_S):
            kq = ktq[ktile // 8]
            klo = (ktile % 8) * P
            ksl = slice(klo, klo + P)
            # both heads' scores share one [128,1024] PSUM tile
            sp = psum1024()
            a = mm(sp[:, 0:512], kq[0:64, ksl], qq[0:64, qls])
            b = mm(sp[:, 512:1024], kq[64:128, ksl], qq[64:128, qls])
            # pin h64 right after h0: the pair streams through disjoint
            # PE row strips concurrently
            _add_dep_helper(b.ins, a.ins, sync=False, reason="pair order")
            # A@V lags three k-tiles behind the scores so its exp()
            # inputs are always long done.
            if len(pending) >= 3:
                pkt, pea = pending.pop(0)
                emit_av(pkt, pea, b)
            ea = e_pool.tile([P, 1024], DTM, tag="ea")
            nc.scalar.activation(ea[:], sp[:], EXP, scale=0.125)
            pending.append((ktile, ea))
        for pkt, pea in pending:
            emit_av(pkt, pea, None)
        # normalize: O[0:64] * (1 / O[64]) broadcast down. Copy O out of
        # PSUM immediately (frees the bank), then run the denominator
        # chain out of SBUF.
        for o_ps, ot in ((o0, ot0), (o1, ot1)):
            osb = rc_pool.tile([65, 512], F32, tag="osb")
            nc.vector.tensor_copy(out=osb[:], in_=o_ps[:])
            bc = psum512()
            mm(bc[0:64, :], ones64_sb[64:65, :], osb[64:65, :])
            rbc = rc_pool.tile([64, 512], F32, tag="rbc")
            nc.vector.reciprocal(out=rbc[:], in_=bc[0:64, :])
            nc.vector.tensor_mul(ot[:, qsl], osb[0:64, :], rbc[:])

    # ---- stage D: partial output projection Y = O_pair @ Wo_pair ---------
    # (no bias: the host adds bo once after summing the partials)
    wo_sb = []
    for hl in range(2):
        st = stg.tile([64, D], F32, tag="wostg")
        nc.sync.dma_start(out=st[:], in_=wop[hl * 64:(hl + 1) * 64, :])
        woh = w_pool.tile([64, D], DTM, tag=f"wo{hl}")
        nc.vector.tensor_copy(out=woh[:], in_=st[:])
        wo_sb.append(woh)
    for qt_i in range(S // P):
        ps = psum512()
        mm(ps[:], ot0[:, qt_i * P:(qt_i + 1) * P], wo_sb[0][:],
           start=True, stop=False)
        mm(ps[:], ot1[:, qt_i * P:(qt_i + 1) * P], wo_sb[1][:],
           start=False, stop=True)
        ysb = y_pool.tile([P, D], F32, tag="y")
        nc.vector.tensor_copy(out=ysb[:], in_=ps[:])
        nc.sync.dma_start(out=out[qt_i * P:(qt_i + 1) * P, :], in_=ysb[:])


def build():
    nc = bacc.Bacc("TRN2", target_bir_lowering=False, debug=False,
                   num_devices=N_CORES)
    io = {}
    for nm, shape in (("xb", [S, D]), ("wqp", [D, P]), ("wkp", [D, P]),
                      ("wvp", [D, P]), ("wop", [P, D]), ("bqp", [P, 1]),
                      ("bkp", [P, 1]), ("bvp", [1, P]), ("ident", [P, P])):
        io[nm] = nc.dram_tensor(nm, shape, F32, kind="ExternalInput").ap()
    io["out"] = nc.dram_tensor("out", [S, D], F32, kind="ExternalOutput").ap()
    with tile.TileContext(nc) as tc:
        with ExitStack() as ctx:
            _emit(ctx, tc, io)
    nc.compile()
    return nc


def make_in_maps(inputs):
    f = lambda a: np.ascontiguousarray(np.asarray(a, dtype=np.float32))
    x = f(inputs["x"])
    Wq, Wk, Wv, Wo = (f(inputs[k]) for k in ("Wq", "Wk", "Wv", "Wo"))
    bq, bk, bv = (f(inputs[k]).reshape(-1) for k in ("bq", "bk", "bv"))
    ident = np.eye(P, dtype=np.float32)
    in_maps = []
    for c in range(N_CORES):
        b, pr = c // 4, c % 4
        cs = slice(pr * P, (pr + 1) * P)
        in_maps.append({
            "xb": x[b],
            "wqp": f(Wq[:, cs]), "wkp": f(Wk[:, cs]), "wvp": f(Wv[:, cs]),
            "wop": f(Wo[cs, :]),
            "bqp": f(bq[cs]).reshape(P, 1), "bkp": f(bk[cs]).reshape(P, 1),
            "bvp": f(bv[cs]).reshape(1, P),
            "ident": ident,
        })
    return in_maps


_CACHE = {}
LAST_EXEC_NS = None


def run(inputs, trace=False):
    global LAST_EXEC_NS
    if "nc" not in _CACHE:
        _CACHE["nc"] = build()
    nc = _CACHE["nc"]
    kw = {}
    if trace:
        import sys, types
        if "antenv.axon_hooks" not in sys.modules:
            sys.path.insert(0, "/root/.axon_site")
            try:
                from trn_agent_boot.trn_boot import _ntff_profile_via_ctypes
                hook = _ntff_profile_via_ctypes("/opt/axon/libaxon_pjrt.so")
                mod = types.ModuleType("antenv.axon_hooks")
                mod.get_axon_ntff_profile_hook = lambda: hook
                mod.set_axon_ntff_profile_hook = lambda h: None
                sys.modules["antenv.axon_hooks"] = mod
            except Exception:
                pass
        kw = dict(trace=True, trace_cores=[0])
    res = run_bass_kernel_spmd(nc, make_in_maps(inputs),
                               core_ids=list(range(N_CORES)), **kw)
    if trace:
        LAST_EXEC_NS = res.exec_time_ns
    bo = np.asarray(inputs["bo"], np.float32).reshape(1, D)
    out = np.empty((B, S, D), np.float32)
    for b in range(B):
        acc = res.results[b * 4][ "out"].astype(np.float32).copy()
        for pr in range(1, 4):
            acc += res.results[b * 4 + pr]["out"]
        out[b] = acc + bo
    return out


def kernel(**inputs) -> np.ndarray:
    return run(inputs, trace=False)


# revision 26
# speedup vs baseline: 1.0030x; 1.0030x over previous
"""Multi-head self-attention Trainium2 Bass kernel (8-core SPMD).

Sharding: tensor-parallel over (batch, head-pair). With B=2 batches and
H=8 heads there are exactly 8 (batch, head-pair) units; core c handles
batch c//4 and heads {2*(c%4), 2*(c%4)+1}. Each core computes Q/K/V for its
two heads over the full sequence, runs attention, and produces the partial
output projection O_pair @ Wo_pair (no bias). The host sums the four
partials per batch and adds the output bias — a cheap numpy reduction.
Per-core weight slices are passed as separate inputs so the program stays
SPMD-uniform.

Layout strategy: activations live transposed in SBUF ([D, S], d on
partitions). Projections then need no weight transposes:
  K^T = Wk^T x^T   (lhsT = Wk chunk, rhs = x^T chunk)
  V   = x Wv       (lhsT = x^T chunk, rhs = Wv chunk)
Scores are computed transposed ([k, q], k on partitions) so softmax's
denominator comes from a ones-column appended to V (row 64 of the attention
output accumulator), and A^T is directly consumable by the A@V matmul.
exp() runs on the scalar engine with the 1/sqrt(dk) folded into its scale.
The normalized per-head outputs O^T are exactly the lhsT the output
projection wants, so no transposes are needed anywhere except on the input x.

Matmul operands are stored as fp16 (10-bit mantissa; measured end-to-end
absmax relative error ~4e-4): this is the true MAC path, so the PE
clock-gate can warm to 2.4 GHz and fast weight load applies. All
accumulation is fp32 in PSUM; softmax denominators/reciprocals are fp32.

The two heads' score matmuls share one [128,1024] PSUM tile and are pinned
adjacent via a scheduler dependency edge, so they stream through disjoint
PE row strips (0-63 / 64-127) concurrently; one exp() covers both. A@V
matmuls lag three k-tiles behind the scores so their exp() inputs are
always ready.
"""

from contextlib import ExitStack

import numpy as np

import concourse.bass as bass
import concourse.tile as tile
from concourse import bacc, mybir
from concourse.bass import _add_dep_helper
from concourse.bass_utils import run_bass_kernel_spmd

N_CORES = 8
B, S, D, H, DK = 2, 4096, 512, 8, 64
P = 128
NT_S = S // P                  # 32 sequence tiles
NT_D = D // P                  # 4 d-model chunks
QC = S // 512                  # 8 query chunks of 512
VW = 2 * 65                    # 130: per-k-tile width of the augmented V
F32 = mybir.dt.float32
F32R = mybir.dt.float32r
F16 = mybir.dt.float16
EXP = mybir.ActivationFunctionType.Exp

# "f16" (10 mantissa bits, 2.4 GHz MAC path + FWL), "f32r" (13 bits but
# pinned at the 1.2 GHz throttled clock), "f32" (exact, 4 cycles/row).
MM_DTYPE = "f16"
DTM = {"f32r": F32R, "f16": F16, "f32": F32}[MM_DTYPE]


def _emit(ctx: ExitStack, tc: tile.TileContext, io: dict):
    nc = tc.nc
    xb = io["xb"]
    wqp, wkp, wvp, wop = io["wqp"], io["wkp"], io["wvp"], io["wop"]
    bqp, bkp, bvp = io["bqp"], io["bkp"], io["bvp"]
    ident = io["ident"]
    out = io["out"]

    mm = nc.tensor.matmul

    # ---- pools ------------------------------------------------------------
    consts = ctx.enter_context(tc.tile_pool(name="consts", bufs=1))
    xt_pool = ctx.enter_context(tc.tile_pool(name="xt", bufs=1))
    qt_pool = ctx.enter_context(tc.tile_pool(name="qt", bufs=1))
    kt_pool = ctx.enter_context(tc.tile_pool(name="kt", bufs=1))
    v_pool = ctx.enter_context(tc.tile_pool(name="v", bufs=1))
    ot_pool = ctx.enter_context(tc.tile_pool(name="ot", bufs=2))
    w_pool = ctx.enter_context(tc.tile_pool(name="w", bufs=1))
    stg = ctx.enter_context(tc.tile_pool(name="stg", bufs=3))
    e_pool = ctx.enter_context(tc.tile_pool(name="e", bufs=8))
    rc_pool = ctx.enter_context(tc.tile_pool(name="rc", bufs=4))
    y_pool = ctx.enter_context(tc.tile_pool(name="y", bufs=3))
    # PSUM: shared [128,1024] pool (3 bufs x 2 banks) + attention
    # accumulators (2 banks). Projections use [0:512] slices of the pool.
    ps_pool = ctx.enter_context(tc.tile_pool(name="ps", bufs=3, space="PSUM"))
    o_pool = ctx.enter_context(tc.tile_pool(name="o", bufs=2, space="PSUM"))

    def psum1024(dt=F32):
        return ps_pool.tile([P, 1024], dt, tag="ps", name="ps")

    def psum512(dt=F32):
        return psum1024(dt)[:, 0:512]

    # ---- constants --------------------------------------------------------
    ident_sb = consts.tile([P, P], F32, tag="ident")
    nc.sync.dma_start(out=ident_sb[:], in_=ident[:])
    ones_f32 = consts.tile([P, 1], F32, tag="ones_f32")
    nc.vector.memset(ones_f32[:], 1.0)
    ones_sb = consts.tile([1, P], DTM, tag="ones")
    nc.vector.tensor_copy(out=ones_sb[:], in_=ones_f32[0:1, 0:1].broadcast_to([1, P]))
    # a f32 ones row living on partition 64 (denominator broadcast lhsT)
    ones64_sb = consts.tile([65, 64], F32, tag="ones64")
    nc.vector.memset(ones64_sb[64:65, :], 1.0)
    # per-partition bias columns for K^T/Q^T (fused into the PSUM->SBUF
    # copies); bv as a [1, 128] row for the rank-1 bias matmul.
    bkT = consts.tile([P, 1], F32, tag="bkT")
    nc.sync.dma_start(out=bkT[:], in_=bkp[:])
    bqT = consts.tile([P, 1], F32, tag="bqT")
    nc.sync.dma_start(out=bqT[:], in_=bqp[:])
    bv_st = consts.tile([1, P], F32, tag="bv_st")
    nc.sync.dma_start(out=bv_st[:], in_=bvp[:])
    bv_sb = consts.tile([1, P], DTM, tag="bv")
    nc.vector.tensor_copy(out=bv_sb[:], in_=bv_st[:])

    # per-core weight slices -> fp16 SBUF tiles
    def load_w(ap, rows, cols, tag):
        st = stg.tile([P, (rows // P) * cols], F32, tag="wstg")
        nc.sync.dma_start(
            out=st[:, :].rearrange("p (dc m) -> p dc m", dc=rows // P),
            in_=ap.rearrange("(dc p) m -> p dc m", p=P),
        )
        t = w_pool.tile([P, (rows // P) * cols], DTM, tag=tag)
        nc.vector.tensor_copy(out=t[:], in_=st[:])
        return t

    # x^T, Q^T, K^T are held as 4 sequence-quarter tiles so dependency
    # tracking (whole-tile granularity) lets projections start as soon as
    # the quarter they need is transposed, and attention as soon as the
    # first K/Q quarters exist.
    SQ = S // 4                 # 1024 columns per quarter
    xTq = [xt_pool.tile([P, NT_D * SQ], DTM, tag="xT", name=f"xT{i}",
                        bufs=4) for i in range(4)]

    def xslice(dc, s0, s1):
        i = s0 // SQ
        return xTq[i][:, dc * SQ + s0 - i * SQ: dc * SQ + s1 - i * SQ]

    # ---- stages A+B interleaved by sequence quarter ----------------------
    # For each quarter: transpose its 8 x-tiles, project its K^T/Q^T
    # chunks and its V k-tiles. Attention on the first query chunk can
    # then start while later quarters are still being produced.
    wq_sb = load_w(wqp, D, P, "wq")
    wk_sb = load_w(wkp, D, P, "wk")
    wv_sb = load_w(wvp, D, P, "wv")
    qtq = [qt_pool.tile([P, SQ], DTM, tag="QT", name=f"QT{i}", bufs=4)
           for i in range(4)]
    ktq = [kt_pool.tile([P, SQ], DTM, tag="KT", name=f"KT{i}", bufs=4)
           for i in range(4)]
    # V (2 heads) with a ones column per head, quartered like K^T:
    # vq[i][:, t*130 + hl*65 + (0..63)] = V[k-tile 8i+t, head hl]
    vq = [v_pool.tile([P, 8 * VW], DTM, tag="vaug", name=f"vq{i}", bufs=4)
          for i in range(4)]

    with tc.tile_pool(name="xn", bufs=6) as xn_pool:
        for i in range(4):
            nc.vector.tensor_copy(
                out=vq[i][:, :].rearrange("p (t h e) -> p t h e",
                                          t=8, h=2)[:, :, :, 64:65],
                in_=ones_f32[:, 0:1].broadcast_to([P, 8, 2, 1]),
            )
            for st in range(8 * i, 8 * i + 8):
                xn = xn_pool.tile([P, D], F32, tag="xn")
                nc.sync.dma_start(out=xn[:], in_=xb[st * P:(st + 1) * P, :])
                tp = psum512()
                for dc in range(NT_D):
                    nc.tensor.transpose(
                        tp[:, dc * P:(dc + 1) * P],
                        xn[:, dc * P:(dc + 1) * P],
                        ident_sb[:],
                    )
                dst_ap = xTq[i][:, :].rearrange("p (dc s) -> p dc s", dc=NT_D)
                so = (st % 8) * P
                nc.vector.tensor_copy(
                    out=dst_ap[:, :, so:so + P],
                    in_=tp[:, :].rearrange("p (dc j) -> p dc j", dc=NT_D),
                )
            for sc in (2 * i, 2 * i + 1):
                for w_sb, dstq, bT in ((wk_sb, ktq, bkT), (wq_sb, qtq, bqT)):
                    ps = psum512()
                    for dc in range(NT_D):
                        mm(ps[:], w_sb[:, dc * P:(dc + 1) * P],
                           xslice(dc, sc * 512, (sc + 1) * 512),
                           start=(dc == 0), stop=(dc == NT_D - 1))
                    so = (sc % 2) * 512
                    nc.vector.tensor_scalar_add(
                        out=dstq[sc // 2][:, so:so + 512], in0=ps[:],
                        scalar1=bT[:],
                    )
            for st in range(8 * i, 8 * i + 8):
                ps = psum512()
                for dc in range(NT_D):
                    mm(ps[:, 0:P], xslice(dc, st * P, (st + 1) * P),
                       wv_sb[:, dc * P:(dc + 1) * P],
                       start=(dc == 0), stop=False)
                mm(ps[:, 0:P], ones_sb[0:1, :], bv_sb[0:1, :],
                   start=False, stop=True)
                dst = vq[i][:, (st % 8) * VW:(st % 8 + 1) * VW]
                dst = dst.rearrange("p (h e) -> p h e", h=2)[:, :, 0:64]
                nc.vector.tensor_copy(
                    out=dst, in_=ps[:, 0:P].rearrange("p (h e) -> p h e", h=2)
                )

    # ---- stage C: attention (+ incremental output projection) -----------
    # load Wo up front so the per-qc partial output projection can overlap
    # the next query chunk's attention
    wo_sb = []
    for hl in range(2):
        st = stg.tile([64, D], F32, tag="wostg")
        nc.sync.dma_start(out=st[:], in_=wop[hl * 64:(hl + 1) * 64, :])
        woh = w_pool.tile([64, D], DTM, tag=f"wo{hl}")
        nc.vector.tensor_copy(out=woh[:], in_=st[:])
        wo_sb.append(woh)
    ot0 = ot_pool.tile([64, S], DTM, tag="OT")
    ot1 = ot_pool.tile([64, S], DTM, tag="OT")
    for qc in range(QC):
        qsl = slice(qc * 512, (qc + 1) * 512)
        o0 = o_pool.tile([65, 512], F32, tag="O")
        o1 = o_pool.tile([65, 512], F32, tag="O")

        def emit_av(ktile, ea, gate):
            va = vq[ktile // 8]
            st_ = (ktile % 8) * VW
            fl = dict(start=(ktile == 0), stop=(ktile == NT_S - 1))
            i0 = mm(o0[:], va[:, st_ + 0 * 65:st_ + 0 * 65 + 65],
                    ea[:, 0:512], **fl)
            i1 = mm(o1[:], va[:, st_ + 1 * 65:st_ + 1 * 65 + 65],
                    ea[:, 512:1024], **fl)
            if gate is not None:
                # order A@V after the next score pair: keeps the paired
                # heads adjacent in the PE stream
                _add_dep_helper(i0.ins, gate.ins, sync=False,
                                reason="attn pipeline order")
                _add_dep_helper(i1.ins, gate.ins, sync=False,
                                reason="attn pipeline order")

        qq = qtq[qc // 2]
        qlo = (qc % 2) * 512
        qls = slice(qlo, qlo + 512)
        pending = []  # [(ktile, ea), ...] not yet AV-emitted
        for ktile in range(NT_S):
            kq = ktq[ktile // 8]
            klo = (ktile % 8) * P
            ksl = slice(klo, klo + P)
            # both heads' scores share one [128,1024] PSUM tile
            sp = psum1024()
            a = mm(sp[:, 0:512], kq[0:64, ksl], qq[0:64, qls])
            b = mm(sp[:, 512:1024], kq[64:128, ksl], qq[64:128, qls])
            # pin h64 right after h0: the pair streams through disjoint
            # PE row strips concurrently
            _add_dep_helper(b.ins, a.ins, sync=False, reason="pair order")
            # A@V lags three k-tiles behind the scores so its exp()
            # inputs are always long done.
            if len(pending) >= 3:
                pkt, pea = pending.pop(0)
                emit_av(pkt, pea, b)
            ea = e_pool.tile([P, 1024], DTM, tag="ea")
            nc.scalar.activation(ea[:], sp[:], EXP, scale=0.125)
            pending.append((ktile, ea))
        for pkt, pea in pending:
            emit_av(pkt, pea, None)
        # normalize: O[0:64] * (1 / O[64]) broadcast down. Copy O out of
        # PSUM immediately (frees the bank), then run the denominator
        # chain out of SBUF.
        for o_ps, ot in ((o0, ot0), (o1, ot1)):
            osb = rc_pool.tile([65, 512], F32, tag="osb")
            nc.vector.tensor_copy(out=osb[:], in_=o_ps[:])
            bc = psum512()
            mm(bc[0:64, :], ones64_sb[64:65, :], osb[64:65, :])
            rbc = rc_pool.tile([64, 512], F32, tag="rbc")
            nc.vector.reciprocal(out=rbc[:], in_=bc[0:64, :])
            nc.vector.tensor_mul(ot[:, qsl], osb[0:64, :], rbc[:])
        # partial output projection for this query chunk (no bias: the
        # host adds bo once after summing the partials)
        for qt_i in range(qc * 4, qc * 4 + 4):
            ps = psum512()
            mm(ps[:], ot0[:, qt_i * P:(qt_i + 1) * P], wo_sb[0][:],
               start=True, stop=False)
            mm(ps[:], ot1[:, qt_i * P:(qt_i + 1) * P], wo_sb[1][:],
               start=False, stop=True)
            ysb = y_pool.tile([P, D], F32, tag="y")
            nc.vector.tensor_copy(out=ysb[:], in_=ps[:])
            nc.sync.dma_start(out=out[qt_i * P:(qt_i + 1) * P, :], in_=ysb[:])


def build():
    nc = bacc.Bacc("TRN2", target_bir_lowering=False, debug=False,
                   num_devices=N_CORES)
    io = {}
    for nm, shape in (("xb", [S, D]), ("wqp", [D, P]), ("wkp", [D, P]),
                      ("wvp", [D, P]), ("wop", [P, D]), ("bqp", [P, 1]),
                      ("bkp", [P, 1]), ("bvp", [1, P]), ("ident", [P, P])):
        io[nm] = nc.dram_tensor(nm, shape, F32, kind="ExternalInput").ap()
    io["out"] = nc.dram_tensor("out", [S, D], F32, kind="ExternalOutput").ap()
    with tile.TileContext(nc) as tc:
        with ExitStack() as ctx:
            _emit(ctx, tc, io)
    nc.compile()
    return nc


def make_in_maps(inputs):
    f = lambda a: np.ascontiguousarray(np.asarray(a, dtype=np.float32))
    x = f(inputs["x"])
    Wq, Wk, Wv, Wo = (f(inputs[k]) for k in ("Wq", "Wk", "Wv", "Wo"))
    bq, bk, bv = (f(inputs[k]).reshape(-1) for k in ("bq", "bk", "bv"))
    ident = np.eye(P, dtype=np.float32)
    in_maps = []
    for c in range(N_CORES):
        b, pr = c // 4, c % 4
        cs = slice(pr * P, (pr + 1) * P)
        in_maps.append({
            "xb": x[b],
            "wqp": f(Wq[:, cs]), "wkp": f(Wk[:, cs]), "wvp": f(Wv[:, cs]),
            "wop": f(Wo[cs, :]),
            "bqp": f(bq[cs]).reshape(P, 1), "bkp": f(bk[cs]).reshape(P, 1),
            "bvp": f(bv[cs]).reshape(1, P),
            "ident": ident,
        })
    return in_maps


_CACHE = {}
LAST_EXEC_NS = None


def run(inputs, trace=False):
    global LAST_EXEC_NS
    if "nc" not in _CACHE:
        _CACHE["nc"] = build()
    nc = _CACHE["nc"]
    kw = {}
    if trace:
        import sys, types
        if "antenv.axon_hooks" not in sys.modules:
            sys.path.insert(0, "/root/.axon_site")
            try:
                from trn_agent_boot.trn_boot import _ntff_profile_via_ctypes
                hook = _ntff_profile_via_ctypes("/opt/axon/libaxon_pjrt.so")
                mod = types.ModuleType("antenv.axon_hooks")
                mod.get_axon_ntff_profile_hook = lambda: hook
                mod.set_axon_ntff_profile_hook = lambda h: None
                sys.modules["antenv.axon_hooks"] = mod
            except Exception:
                pass
        kw = dict(trace=True, trace_cores=[0])
    res = run_bass_kernel_spmd(nc, make_in_maps(inputs),
                               core_ids=list(range(N_CORES)), **kw)
    if trace:
        LAST_EXEC_NS = res.exec_time_ns
    bo = np.asarray(inputs["bo"], np.float32).reshape(1, D)
    out = np.empty((B, S, D), np.float32)
    for b in range(B):
        acc = res.results[b * 4][ "out"].astype(np.float32).copy()
        for pr in range(1, 4):
            acc += res.results[b * 4 + pr]["out"]
        out[b] = acc + bo
    return out


def kernel(**inputs) -> np.ndarray:
    return run(inputs, trace=False)
